# revision 14
# baseline (speedup 1.0000x reference)
"""Distributed Trainium2 kernel for AutoRegressiveGlobalSelfAttention.

B=2, S=2048, D=1024, H=16 (head_dim 64), causal, no 1/sqrt(hd) scale.
Returns (output [B,S,D], proba [B,H,S,S]) like the reference nn.Module.

Sharding: tensor-parallel over heads. Core c owns heads {2c, 2c+1} for both
batch elements. Per core:
  - QKV projections for its 128 output channels (f32r matmuls on the PE)
  - attention in transposed layout S^T[k,q]: exp on ScalarE, causal diagonal
    via affine_select, softmax denominator via a ones-column appended to V
    (the context matmul computes row sums for free)
  - context comes out channel-major [ch, tok] == the AllToAll input layout
  - one AllToAll per batch element (converts head-sharding -> token-sharding,
    256 tokens per core per batch); batch 0's collective and output
    projection overlap batch 1's attention
  - output projection with full Wo^T on the local token slices
proba is written to DRAM transposed ([k, q]); the causal upper triangle is
never written (output buffers are pre-zeroed by the runtime); the host
restores [q, k] order when assembling the full array.
"""

import numpy as np

import concourse.bacc as bacc
import concourse.tile as tile
import concourse.mybir as mybir
import concourse.bass_utils as bass_utils
from concourse import masks

B, S, D, H = 2, 2048, 1024, 16
HD = D // H          # 64
N_CORES = 8
HPC = H // N_CORES   # heads per core = 2
CPC = HPC * HD       # channels per core = 128
T = B * S            # 4096 flattened tokens
SPC = S // N_CORES   # tokens per core per batch after a2a = 256
STRIP = 512          # q-strip width
N_STRIP = S // STRIP  # 4 strips per batch element
NJT = S // 128       # 16 k-tiles of 128 per batch element

F32 = mybir.dt.float32
F32R = mybir.dt.float32r
EXP = mybir.ActivationFunctionType.Exp

_COMPILED = None


def _build():
    nc = bacc.Bacc(
        "TRN2",
        target_bir_lowering=False,
        debug=False,
        enable_asserts=True,
        num_devices=N_CORES,
    )
    # ---- I/O ----
    hst = nc.dram_tensor("hst", [D, T], F32, kind="ExternalInput").ap()       # hs^T
    wqt = nc.dram_tensor("wqt", [D, CPC], F32, kind="ExternalInput").ap()     # Wq_c^T
    wkt = nc.dram_tensor("wkt", [D, CPC], F32, kind="ExternalInput").ap()
    wvt = nc.dram_tensor("wvt", [D, CPC], F32, kind="ExternalInput").ap()
    wot = nc.dram_tensor("wot", [D, D], F32, kind="ExternalInput").ap()       # Wo^T
    bo = nc.dram_tensor("bo", [1, D], F32, kind="ExternalInput").ap()
    # out rows: [batch0 tokens (256), batch1 tokens (256)]
    out = nc.dram_tensor("out", [B * SPC, D], F32, kind="ExternalOutput").ap()
    # proba[b*HPC + h][k][q]  (transposed per head-batch)
    proba = nc.dram_tensor("proba", [B * HPC, N_STRIP, S, STRIP], F32,
                           kind="ExternalOutput").ap()

    hst_r = hst.bitcast(F32R)
    wot_r = wot.bitcast(F32R)

    with tile.TileContext(nc) as tc:
        with tc.tile_pool(name="const", bufs=1) as constp, \
             tc.tile_pool(name="dram", bufs=1, space="DRAM") as dram:
            ident = constp.tile([128, 128], F32)
            masks.make_identity(nc, ident[:])
            ones_row = constp.tile([1, 128], F32R)
            nc.vector.memset(ones_row.bitcast(F32)[:], 1.0)

            # Wo^T + bias SBUF space (loaded later, off the startup path)
            wo_sb = constp.tile([128, 8 * D], F32R, name="wo_sb")
            bo_sb = constp.tile([1, D], F32)
            bo_b = constp.tile([128, D], F32)

            a2a = []
            for b in range(B):
                ain = dram.tile([N_CORES, CPC, SPC], F32, name=f"a2a_in{b}")
                aout = dram.tile([N_CORES, CPC, SPC], F32, name=f"a2a_out{b}")
                a2a.append((ain, aout))

            # ---- projections for both batches ----
            qt, kt, vaug = [], [], []
            with tc.tile_pool(name="qkvp", bufs=1) as qkv:
                for b in range(B):
                    qt.append(qkv.tile([128, S], F32R, name=f"qt{b}"))
                    kt.append(qkv.tile([128, S], F32R, name=f"kt{b}"))
                    vaug.append([qkv.tile([128, NJT * 65], F32R, name=f"vaug{b}{h}")
                                 for h in range(HPC)])
                with tc.tile_pool(name="hstp", bufs=8) as hsp, \
                     tc.tile_pool(name="prps", bufs=4, space="PSUM") as prps, \
                     tc.tile_pool(name="vtmp", bufs=2) as vtmp:
                    # qkv weights first on the sync queue, shared by both passes
                    w_sb = vtmp.tile([128, 3 * 8 * CPC], F32R, name="w_sb", bufs=1)
                    for i, w in enumerate((wqt, wkt, wvt)):
                        for kc in range(8):
                            nc.sync.dma_start(
                                w_sb[:, (i * 8 + kc) * CPC:(i * 8 + kc + 1) * CPC],
                                w.bitcast(F32R)[kc * 128:(kc + 1) * 128, :],
                            )
                    for b in range(B):
                        _proj_batch(nc, b, hst_r, w_sb, ident, hsp, prps, vtmp,
                                    qt[b], kt[b], vaug[b])
                    # Wo^T + bias loads: emitted after proj, gpsimd queue
                    for kc in range(8):
                        nc.gpsimd.dma_start(
                            wo_sb[:, kc * D:(kc + 1) * D],
                            wot_r[kc * 128:(kc + 1) * 128, :],
                        )
                    nc.gpsimd.dma_start(bo_sb[:], bo[:])
                    nc.gpsimd.partition_broadcast(bo_b[:], bo_sb[:])

                # ---- attention + per-batch a2a + output projection ----
                with tc.tile_pool(name="ptp", bufs=38) as ptp, \
                     tc.tile_pool(name="smp", bufs=2) as smp, \
                     tc.tile_pool(name="mmps", bufs=2, space="PSUM") as mmps:
                    _attention_batch(nc, 0, qt[0], kt[0], vaug[0], ones_row,
                                     proba, a2a[0][0], ptp, smp, mmps)
                    nc.gpsimd.collective_compute(
                        "AllToAll", mybir.AluOpType.bypass,
                        replica_groups=[list(range(N_CORES))],
                        ins=[a2a[0][0].opt()], outs=[a2a[0][1].opt()],
                    )
                    _attention_batch(nc, 1, qt[1], kt[1], vaug[1], ones_row,
                                     proba, a2a[1][0], ptp, smp, mmps)
                    _outproj_batch(nc, 0, a2a[0][1], wo_sb, bo_b, out,
                                   smp, mmps)
                    nc.gpsimd.collective_compute(
                        "AllToAll", mybir.AluOpType.bypass,
                        replica_groups=[list(range(N_CORES))],
                        ins=[a2a[1][0].opt()], outs=[a2a[1][1].opt()],
                    )
                    _outproj_batch(nc, 1, a2a[1][1], wo_sb, bo_b, out,
                                   smp, mmps)

    nc.compile()
    return nc


def _proj_batch(nc, b, hst_r, w_sb, ident, hsp, prps, vtmp, qt, kt, vaug):
    hs = []
    for kc in range(8):
        t_ = hsp.tile([128, S], F32R, tag="hst", name=f"hs{b}{kc}")
        nc.sync.dma_start(t_[:], hst_r[kc * 128:(kc + 1) * 128,
                                        b * S:(b + 1) * S])
        hs.append(t_)

    vt = vtmp.tile([128, S], F32, name=f"vt{b}", bufs=1)
    for i, dst in enumerate((qt, kt, vt)):
        for n in range(S // 512):
            ps = prps.tile([128, 512], F32, tag="prps", name=f"pp{b}")
            for kc in range(8):
                nc.tensor.matmul(
                    ps[:],
                    w_sb[:, (i * 8 + kc) * CPC:(i * 8 + kc + 1) * CPC],
                    hs[kc][:, n * 512:(n + 1) * 512],
                    start=(kc == 0),
                    stop=(kc == 7),
                )
            if i < 2:
                nc.scalar.copy(dst[:, n * 512:(n + 1) * 512], ps[:])
            else:
                nc.vector.tensor_copy(dst[:, n * 512:(n + 1) * 512], ps[:])

    # transpose v^T -> token-major v, pack into v_aug with ones col
    for h in range(HPC):
        nc.vector.memset(
            vaug[h].bitcast(F32).rearrange("p (j c) -> p j c", c=65)[:, :, 64],
            1.0,
        )
    for jt in range(NJT):
        pst = prps.tile([128, 128], F32, tag="pst", bufs=2, name=f"pst{b}")
        nc.tensor.transpose(pst[:], vt[:, jt * 128:(jt + 1) * 128], ident[:])
        for h in range(HPC):
            nc.vector.tensor_copy(
                vaug[h][:, jt * 65: jt * 65 + 64],
                pst[:, h * 64:(h + 1) * 64],
            )


def _attention_batch(nc, b, qt, kt, vaug, ones_row, proba, a2a_in, ptp, smp, mmps):
    for i4 in range(N_STRIP):
        q0 = i4 * STRIP
        njs = (q0 + STRIP) // 128  # j-tiles needed for this strip
        ctx_ps = [mmps.tile([65, STRIP], F32, tag=f"cps{h}", name=f"cps{b}{h}")
                  for h in range(HPC)]
        pts = [[None] * njs for _ in range(HPC)]
        for j in range(njs):
            scs = []
            for h in range(HPC):  # adjacent emission -> PE row-group packing
                sc = mmps.tile([128, STRIP], F32, tag=f"sps{h}", name=f"sc{b}{h}")
                nc.tensor.matmul(
                    sc[:],
                    kt[h * 64:(h + 1) * 64, j * 128:(j + 1) * 128],
                    qt[h * 64:(h + 1) * 64, q0:q0 + STRIP],
                    start=True, stop=True,
                    tile_position=(64 * h, 0),
                )
                scs.append(sc)
            for h in range(HPC):
                pt = ptp.tile([128, STRIP], F32R, tag="pt", name=f"pt{b}{h}")
                nc.scalar.activation(pt[:], scs[h][:], EXP)
                if j * 128 + 127 >= q0:  # diagonal block: causal mask
                    nc.gpsimd.affine_select(
                        out=pt[:], in_=pt[:],
                        compare_op=mybir.AluOpType.is_ge,
                        fill=0.0,
                        base=q0 - j * 128,
                        pattern=[[1, STRIP]],
                        channel_multiplier=-1,
                    )
                pts[h][j] = pt
            for h in range(HPC):
                nc.tensor.matmul(
                    ctx_ps[h][:],
                    vaug[h][:, j * 65:(j + 1) * 65],
                    pts[h][j][:],
                    start=(j == 0), stop=(j == njs - 1),
                )

        for h in range(HPC):
            # 1/denom = exp(-ln(denom)) on ScalarE, then K=1 f32r PE broadcast
            lnd = smp.tile([1, STRIP], F32, tag="lnd", name=f"ld{b}{h}", bufs=2)
            nc.scalar.activation(lnd[:], ctx_ps[h][64:65, :],
                                 mybir.ActivationFunctionType.Ln)
            recip = smp.tile([1, STRIP], F32R, tag="recip", name=f"rc{b}{h}", bufs=2)
            nc.scalar.activation(recip[:], lnd[:], EXP, scale=-1.0)
            rb_ps = mmps.tile([128, STRIP], F32, tag=f"sps{h}", name=f"rp{b}{h}")
            nc.tensor.matmul(rb_ps[:], ones_row[:], recip[:], start=True, stop=True)
            rb_sb = smp.tile([128, STRIP], F32, tag="rb_sb", name=f"rs{b}{h}", bufs=2)
            nc.vector.tensor_copy(rb_sb[:], rb_ps[:])
            bh = b * HPC + h
            for j in range(njs):
                pt = pts[h][j]
                nc.vector.tensor_mul(pt[:], pt[:], rb_sb[:])
                nc.sync.dma_start(
                    proba[bh, i4, j * 128:(j + 1) * 128, :],
                    pt.bitcast(F32)[:],
                )
            # normalize ctx, ship straight into the a2a input slots
            ctx_sb = smp.tile([64, STRIP], F32, tag="ctx_sb", name=f"cs{b}{h}", bufs=2)
            nc.vector.tensor_mul(ctx_sb[:], ctx_ps[h][0:64, :], rb_sb[0:64, :])
            for half in range(2):  # strip covers two 256-token a2a slices
                d = 2 * i4 + half
                nc.sync.dma_start(
                    a2a_in[d, h * 64:(h + 1) * 64, :],
                    ctx_sb[:, half * SPC:(half + 1) * SPC],
                )


def _outproj_batch(nc, b, a2a_out, wo_sb, bo_b, out, smp, mmps):
    ctxf = smp.tile([128, 8 * SPC], F32R, tag="ctxf", name=f"ctxf{b}", bufs=1)
    for kc in range(N_CORES):
        nc.sync.dma_start(
            ctxf[:, kc * SPC:(kc + 1) * SPC],
            a2a_out[kc].bitcast(F32R),
        )
    for t in range(SPC // 128):  # 2 token tiles
        for n in range(2):       # 2 x 512 output channels
            ps = mmps.tile([128, 512], F32, tag=f"sps{n}", name=f"op{b}{n}")
            for kc in range(8):
                nc.tensor.matmul(
                    ps[:],
                    ctxf[:, kc * SPC + t * 128: kc * SPC + (t + 1) * 128],
                    wo_sb[:, kc * D + n * 512: kc * D + (n + 1) * 512],
                    start=(kc == 0),
                    stop=(kc == 7),
                )
            ot = smp.tile([128, 512], F32, tag="osb_t", name=f"ot{b}{n}", bufs=2)
            nc.vector.tensor_add(ot[:], ps[:], bo_b[:, n * 512:(n + 1) * 512])
            nc.sync.dma_start(
                out[b * SPC + t * 128: b * SPC + (t + 1) * 128,
                    n * 512:(n + 1) * 512],
                ot[:],
            )


def _get_nc():
    global _COMPILED
    if _COMPILED is None:
        _COMPILED = _build()
    return _COMPILED


def _prep_in_maps(inputs):
    hidden_states = np.asarray(inputs["hidden_states"], dtype=np.float32)
    Wq = np.asarray(inputs["Wq"], dtype=np.float32)
    Wk = np.asarray(inputs["Wk"], dtype=np.float32)
    Wv = np.asarray(inputs["Wv"], dtype=np.float32)
    Wo = np.asarray(inputs["Wo"], dtype=np.float32)
    bo = np.asarray(inputs["bo"], dtype=np.float32)

    hst = np.ascontiguousarray(hidden_states.reshape(T, D).T)   # [D, T]
    wot = np.ascontiguousarray(Wo.T)                            # [D, D]
    bo2 = bo.reshape(1, D)
    in_maps = []
    for c in range(N_CORES):
        sl = slice(c * CPC, (c + 1) * CPC)
        in_maps.append({
            "hst": hst,
            "wqt": np.ascontiguousarray(Wq[sl, :].T),
            "wkt": np.ascontiguousarray(Wk[sl, :].T),
            "wvt": np.ascontiguousarray(Wv[sl, :].T),
            "wot": wot,
            "bo": bo2,
        })
    return in_maps


def kernel(hidden_states, attention_mask, Wq, Wk, Wv, Wo, bo):
    in_maps = _prep_in_maps({
        "hidden_states": hidden_states, "Wq": Wq, "Wk": Wk,
        "Wv": Wv, "Wo": Wo, "bo": bo,
    })
    nc = _get_nc()
    res = bass_utils.run_bass_kernel_spmd(
        nc, in_maps, core_ids=list(range(N_CORES))
    )

    # out rows per core: [b0 tokens 256c..256c+256, b1 tokens 256c..]
    output = np.empty((B, S, D), dtype=np.float32)
    for c in range(N_CORES):
        o = res.results[c]["out"]
        for b in range(B):
            output[b, c * SPC:(c + 1) * SPC] = o[b * SPC:(b + 1) * SPC]
    proba = np.empty((B, H, S, S), dtype=np.float32)
    for c in range(N_CORES):
        pr = res.results[c]["proba"]  # [B*HPC, N_STRIP, S(k), STRIP(q)]
        for b in range(B):
            for h in range(HPC):
                blk = pr[b * HPC + h]
                for i4 in range(N_STRIP):
                    proba[b, HPC * c + h, i4 * STRIP:(i4 + 1) * STRIP, :] = \
                        blk[i4].T
    return output, proba


# revision 15
# speedup vs baseline: 1.0427x; 1.0427x over previous
"""Distributed Trainium2 kernel for AutoRegressiveGlobalSelfAttention.

B=2, S=2048, D=1024, H=16 (head_dim 64), causal, no 1/sqrt(hd) scale.
Returns (output [B,S,D], proba [B,H,S,S]) like the reference nn.Module.

Sharding: tensor-parallel over heads. Core c owns heads {2c, 2c+1} for both
batch elements. Per core:
  - QKV projections for its 128 output channels (f32r matmuls on the PE)
  - attention in transposed layout S^T[k,q]: exp on ScalarE, causal diagonal
    via affine_select, softmax denominator via a ones-column appended to V
    (the context matmul computes row sums for free)
  - context comes out channel-major [ch, tok] == the AllToAll input layout
  - one AllToAll per batch element (converts head-sharding -> token-sharding,
    256 tokens per core per batch); batch 0's collective and output
    projection overlap batch 1's attention
  - output projection with full Wo^T on the local token slices
proba is written to DRAM transposed ([k, q]); the causal upper triangle is
never written (output buffers are pre-zeroed by the runtime); the host
restores [q, k] order when assembling the full array.
"""

import numpy as np

import concourse.bacc as bacc
import concourse.tile as tile
import concourse.mybir as mybir
import concourse.bass_utils as bass_utils
from concourse import masks

B, S, D, H = 2, 2048, 1024, 16
HD = D // H          # 64
N_CORES = 8
HPC = H // N_CORES   # heads per core = 2
CPC = HPC * HD       # channels per core = 128
T = B * S            # 4096 flattened tokens
SPC = S // N_CORES   # tokens per core per batch after a2a = 256
STRIP = 512          # q-strip width
N_STRIP = S // STRIP  # 4 strips per batch element
NJT = S // 128       # 16 k-tiles of 128 per batch element

F32 = mybir.dt.float32
F32R = mybir.dt.float32r
EXP = mybir.ActivationFunctionType.Exp

_COMPILED = None


def _build():
    nc = bacc.Bacc(
        "TRN2",
        target_bir_lowering=False,
        debug=False,
        enable_asserts=True,
        num_devices=N_CORES,
    )
    # ---- I/O ----
    hst = nc.dram_tensor("hst", [D, T], F32, kind="ExternalInput").ap()       # hs^T
    wqt = nc.dram_tensor("wqt", [D, CPC], F32, kind="ExternalInput").ap()     # Wq_c^T
    wkt = nc.dram_tensor("wkt", [D, CPC], F32, kind="ExternalInput").ap()
    wvt = nc.dram_tensor("wvt", [D, CPC], F32, kind="ExternalInput").ap()
    wot = nc.dram_tensor("wot", [D, D], F32, kind="ExternalInput").ap()       # Wo^T
    bo = nc.dram_tensor("bo", [1, D], F32, kind="ExternalInput").ap()
    # out rows: [batch0 tokens (256), batch1 tokens (256)]
    out = nc.dram_tensor("out", [B * SPC, D], F32, kind="ExternalOutput").ap()
    # proba[b*HPC + h][k][q]  (transposed per head-batch)
    proba = nc.dram_tensor("proba", [B * HPC, N_STRIP, S, STRIP], F32,
                           kind="ExternalOutput").ap()

    hst_r = hst.bitcast(F32R)
    wot_r = wot.bitcast(F32R)

    with tile.TileContext(nc) as tc:
        with tc.tile_pool(name="const", bufs=1) as constp, \
             tc.tile_pool(name="dram", bufs=1, space="DRAM") as dram:
            ident = constp.tile([128, 128], F32)
            masks.make_identity(nc, ident[:])
            ones_row = constp.tile([1, 128], F32R)
            nc.vector.memset(ones_row.bitcast(F32)[:], 1.0)

            # Wo^T + bias SBUF space (loaded later, off the startup path)
            wo_sb = constp.tile([128, 8 * D], F32R, name="wo_sb")
            bo_sb = constp.tile([1, D], F32)
            bo_b = constp.tile([128, D], F32)

            a2a = []
            for b in range(B):
                ain = dram.tile([N_CORES, CPC, SPC], F32, name=f"a2a_in{b}")
                aout = dram.tile([N_CORES, CPC, SPC], F32, name=f"a2a_out{b}")
                a2a.append((ain, aout))

            # ---- projections for both batches ----
            qt, kt, vaug = [], [], []
            with tc.tile_pool(name="qkvp", bufs=1) as qkv:
                for b in range(B):
                    qt.append(qkv.tile([128, S], F32R, name=f"qt{b}"))
                    kt.append(qkv.tile([128, S], F32R, name=f"kt{b}"))
                    vaug.append([qkv.tile([128, NJT * 65], F32R, name=f"vaug{b}{h}")
                                 for h in range(HPC)])
                with tc.tile_pool(name="hstp", bufs=8) as hsp, \
                     tc.tile_pool(name="prps", bufs=4, space="PSUM") as prps, \
                     tc.tile_pool(name="vtmp", bufs=2) as vtmp:
                    # qkv weights first on the sync queue, shared by both passes
                    w_sb = vtmp.tile([128, 3 * 8 * CPC], F32R, name="w_sb", bufs=1)
                    for i, w in enumerate((wqt, wkt, wvt)):
                        for kc in range(8):
                            nc.sync.dma_start(
                                w_sb[:, (i * 8 + kc) * CPC:(i * 8 + kc + 1) * CPC],
                                w.bitcast(F32R)[kc * 128:(kc + 1) * 128, :],
                            )
                    for b in range(B):
                        _proj_batch(nc, b, hst_r, w_sb, ident, hsp, prps, vtmp,
                                    qt[b], kt[b], vaug[b])
                    # Wo^T + bias loads: emitted after proj, gpsimd queue
                    for kc in range(8):
                        nc.gpsimd.dma_start(
                            wo_sb[:, kc * D:(kc + 1) * D],
                            wot_r[kc * 128:(kc + 1) * 128, :],
                        )
                    nc.gpsimd.dma_start(bo_sb[:], bo[:])
                    nc.gpsimd.partition_broadcast(bo_b[:], bo_sb[:])

                # ---- attention + per-batch a2a + output projection ----
                with tc.tile_pool(name="ptp", bufs=38) as ptp, \
                     tc.tile_pool(name="smp", bufs=2) as smp, \
                     tc.tile_pool(name="mmps", bufs=2, space="PSUM") as mmps:
                    _attention_batch(nc, 0, qt[0], kt[0], vaug[0], ones_row,
                                     proba, a2a[0][0], ptp, smp, mmps)
                    nc.gpsimd.collective_compute(
                        "AllToAll", mybir.AluOpType.bypass,
                        replica_groups=[list(range(N_CORES))],
                        ins=[a2a[0][0].opt()], outs=[a2a[0][1].opt()],
                    )
                    _attention_batch(nc, 1, qt[1], kt[1], vaug[1], ones_row,
                                     proba, a2a[1][0], ptp, smp, mmps)
                    _outproj_batch(nc, 0, a2a[0][1], wo_sb, bo_b, out,
                                   smp, mmps)
                    nc.gpsimd.collective_compute(
                        "AllToAll", mybir.AluOpType.bypass,
                        replica_groups=[list(range(N_CORES))],
                        ins=[a2a[1][0].opt()], outs=[a2a[1][1].opt()],
                    )
                    _outproj_batch(nc, 1, a2a[1][1], wo_sb, bo_b, out,
                                   smp, mmps)

    nc.compile()
    return nc


def _proj_batch(nc, b, hst_r, w_sb, ident, hsp, prps, vtmp, qt, kt, vaug):
    hs = []
    for kc in range(8):
        t_ = hsp.tile([128, S], F32R, tag="hst", name=f"hs{b}{kc}")
        nc.sync.dma_start(t_[:], hst_r[kc * 128:(kc + 1) * 128,
                                        b * S:(b + 1) * S])
        hs.append(t_)

    vt = vtmp.tile([128, S], F32, name=f"vt{b}", bufs=1)
    for i, dst in enumerate((qt, kt, vt)):
        for n in range(S // 512):
            ps = prps.tile([128, 512], F32, tag="prps", name=f"pp{b}")
            for kc in range(8):
                nc.tensor.matmul(
                    ps[:],
                    w_sb[:, (i * 8 + kc) * CPC:(i * 8 + kc + 1) * CPC],
                    hs[kc][:, n * 512:(n + 1) * 512],
                    start=(kc == 0),
                    stop=(kc == 7),
                )
            if i < 2:
                nc.scalar.copy(dst[:, n * 512:(n + 1) * 512], ps[:])
            else:
                nc.vector.tensor_copy(dst[:, n * 512:(n + 1) * 512], ps[:])

    # transpose v^T -> token-major v, pack into v_aug with ones col
    for h in range(HPC):
        nc.vector.memset(
            vaug[h].bitcast(F32).rearrange("p (j c) -> p j c", c=65)[:, :, 64],
            1.0,
        )
    for jt in range(NJT):
        pst = prps.tile([128, 128], F32, tag="pst", bufs=2, name=f"pst{b}")
        nc.tensor.transpose(pst[:], vt[:, jt * 128:(jt + 1) * 128], ident[:])
        for h in range(HPC):
            nc.vector.tensor_copy(
                vaug[h][:, jt * 65: jt * 65 + 64],
                pst[:, h * 64:(h + 1) * 64],
            )


def _attention_batch(nc, b, qt, kt, vaug, ones_row, proba, a2a_in, ptp, smp, mmps):
    for i4 in range(N_STRIP):
        q0 = i4 * STRIP
        njs = (q0 + STRIP) // 128  # j-tiles needed for this strip
        ctx_ps = [mmps.tile([65, STRIP], F32, tag=f"cps{h}", name=f"cps{b}{h}")
                  for h in range(HPC)]
        pts = [[None] * njs for _ in range(HPC)]
        for j in range(njs):
            # causal trim: block j only has valid q >= 128j, i.e. local
            # columns >= t0. Head columns are never computed, read, or
            # written (DRAM is pre-zeroed).
            t0 = max(0, 128 * (j - 4 * i4))
            scs = []
            for h in range(HPC):  # adjacent emission -> PE row-group packing
                sc = mmps.tile([128, STRIP], F32, tag=f"sps{h}", name=f"sc{b}{h}")
                nc.tensor.matmul(
                    sc[:, t0:],
                    kt[h * 64:(h + 1) * 64, j * 128:(j + 1) * 128],
                    qt[h * 64:(h + 1) * 64, q0 + t0:q0 + STRIP],
                    start=True, stop=True,
                    tile_position=(64 * h, 0),
                )
                scs.append(sc)
            for h in range(HPC):
                pt = ptp.tile([128, STRIP], F32R, tag="pt", name=f"pt{b}{h}")
                nc.scalar.activation(pt[:, t0:], scs[h][:, t0:], EXP)
                if j * 128 + 127 >= q0:  # diagonal 128-wide triangle only
                    nc.gpsimd.affine_select(
                        out=pt[:, t0:t0 + 128], in_=pt[:, t0:t0 + 128],
                        compare_op=mybir.AluOpType.is_ge,
                        fill=0.0,
                        base=0,
                        pattern=[[1, 128]],
                        channel_multiplier=-1,
                    )
                pts[h][j] = pt
            for h in range(HPC):
                nc.tensor.matmul(
                    ctx_ps[h][:, t0:],
                    vaug[h][:, j * 65:(j + 1) * 65],
                    pts[h][j][:, t0:],
                    start=(j == 0), stop=(j == njs - 1),
                )

        for h in range(HPC):
            # 1/denom on DVE (approx), then K=1 f32r PE broadcast
            den = smp.tile([1, STRIP], F32, tag="den", name=f"dn{b}{h}", bufs=2)
            nc.vector.tensor_copy(den[:], ctx_ps[h][64:65, :])
            rcp = smp.tile([1, STRIP], F32, tag="rcp", name=f"rf{b}{h}", bufs=2)
            nc.vector.reciprocal_approx_fast(out=rcp[:], in_=den[:])
            recip = smp.tile([1, STRIP], F32R, tag="recip", name=f"rc{b}{h}", bufs=2)
            nc.vector.tensor_copy(recip[:], rcp[:])
            rb_ps = mmps.tile([128, STRIP], F32, tag=f"sps{h}", name=f"rp{b}{h}")
            nc.tensor.matmul(rb_ps[:], ones_row[:], recip[:], start=True, stop=True)
            rb_sb = smp.tile([128, STRIP], F32, tag="rb_sb", name=f"rs{b}{h}", bufs=2)
            nc.vector.tensor_copy(rb_sb[:], rb_ps[:])
            bh = b * HPC + h
            for j in range(njs):
                t0 = max(0, 128 * (j - 4 * i4))
                pt = pts[h][j]
                nc.vector.tensor_mul(pt[:, t0:], pt[:, t0:], rb_sb[:, t0:])
                nc.sync.dma_start(
                    proba[bh, i4, j * 128:(j + 1) * 128, t0:],
                    pt.bitcast(F32)[:, t0:],
                )
            # normalize ctx, ship straight into the a2a input slots
            ctx_sb = smp.tile([64, STRIP], F32, tag="ctx_sb", name=f"cs{b}{h}", bufs=2)
            nc.vector.tensor_mul(ctx_sb[:], ctx_ps[h][0:64, :], rb_sb[0:64, :])
            for half in range(2):  # strip covers two 256-token a2a slices
                d = 2 * i4 + half
                nc.sync.dma_start(
                    a2a_in[d, h * 64:(h + 1) * 64, :],
                    ctx_sb[:, half * SPC:(half + 1) * SPC],
                )


def _outproj_batch(nc, b, a2a_out, wo_sb, bo_b, out, smp, mmps):
    ctxf = smp.tile([128, 8 * SPC], F32R, tag="ctxf", name=f"ctxf{b}", bufs=1)
    for kc in range(N_CORES):
        nc.sync.dma_start(
            ctxf[:, kc * SPC:(kc + 1) * SPC],
            a2a_out[kc].bitcast(F32R),
        )
    for t in range(SPC // 128):  # 2 token tiles
        for n in range(2):       # 2 x 512 output channels
            ps = mmps.tile([128, 512], F32, tag=f"sps{n}", name=f"op{b}{n}")
            for kc in range(8):
                nc.tensor.matmul(
                    ps[:],
                    ctxf[:, kc * SPC + t * 128: kc * SPC + (t + 1) * 128],
                    wo_sb[:, kc * D + n * 512: kc * D + (n + 1) * 512],
                    start=(kc == 0),
                    stop=(kc == 7),
                )
            ot = smp.tile([128, 512], F32, tag="osb_t", name=f"ot{b}{n}", bufs=2)
            nc.vector.tensor_add(ot[:], ps[:], bo_b[:, n * 512:(n + 1) * 512])
            nc.sync.dma_start(
                out[b * SPC + t * 128: b * SPC + (t + 1) * 128,
                    n * 512:(n + 1) * 512],
                ot[:],
            )


def _get_nc():
    global _COMPILED
    if _COMPILED is None:
        _COMPILED = _build()
    return _COMPILED


def _prep_in_maps(inputs):
    hidden_states = np.asarray(inputs["hidden_states"], dtype=np.float32)
    Wq = np.asarray(inputs["Wq"], dtype=np.float32)
    Wk = np.asarray(inputs["Wk"], dtype=np.float32)
    Wv = np.asarray(inputs["Wv"], dtype=np.float32)
    Wo = np.asarray(inputs["Wo"], dtype=np.float32)
    bo = np.asarray(inputs["bo"], dtype=np.float32)

    hst = np.ascontiguousarray(hidden_states.reshape(T, D).T)   # [D, T]
    wot = np.ascontiguousarray(Wo.T)                            # [D, D]
    bo2 = bo.reshape(1, D)
    in_maps = []
    for c in range(N_CORES):
        sl = slice(c * CPC, (c + 1) * CPC)
        in_maps.append({
            "hst": hst,
            "wqt": np.ascontiguousarray(Wq[sl, :].T),
            "wkt": np.ascontiguousarray(Wk[sl, :].T),
            "wvt": np.ascontiguousarray(Wv[sl, :].T),
            "wot": wot,
            "bo": bo2,
        })
    return in_maps


def kernel(hidden_states, attention_mask, Wq, Wk, Wv, Wo, bo):
    in_maps = _prep_in_maps({
        "hidden_states": hidden_states, "Wq": Wq, "Wk": Wk,
        "Wv": Wv, "Wo": Wo, "bo": bo,
    })
    nc = _get_nc()
    res = bass_utils.run_bass_kernel_spmd(
        nc, in_maps, core_ids=list(range(N_CORES))
    )

    # out rows per core: [b0 tokens 256c..256c+256, b1 tokens 256c..]
    output = np.empty((B, S, D), dtype=np.float32)
    for c in range(N_CORES):
        o = res.results[c]["out"]
        for b in range(B):
            output[b, c * SPC:(c + 1) * SPC] = o[b * SPC:(b + 1) * SPC]
    proba = np.empty((B, H, S, S), dtype=np.float32)
    for c in range(N_CORES):
        pr = res.results[c]["proba"]  # [B*HPC, N_STRIP, S(k), STRIP(q)]
        for b in range(B):
            for h in range(HPC):
                blk = pr[b * HPC + h]
                for i4 in range(N_STRIP):
                    proba[b, HPC * c + h, i4 * STRIP:(i4 + 1) * STRIP, :] = \
                        blk[i4].T
    return output, proba


# revision 17
# speedup vs baseline: 1.1562x; 1.1089x over previous
"""Distributed Trainium2 kernel for AutoRegressiveGlobalSelfAttention.

B=2, S=2048, D=1024, H=16 (head_dim 64), causal, no 1/sqrt(hd) scale.
Returns (output [B,S,D], proba [B,H,S,S]) like the reference nn.Module.

Sharding: tensor-parallel over heads. Core c owns heads {2c, 2c+1} for both
batch elements. Per core:
  - QKV projections for its 128 output channels (f32r matmuls on the PE)
  - attention in transposed layout S^T[k,q]: exp on ScalarE, causal diagonal
    via affine_select, softmax denominator via a ones-column appended to V
    (the context matmul computes row sums for free); blocks strictly above
    the diagonal are never computed (causal trim)
  - context comes out channel-major [ch, tok] == the AllToAll input layout
  - AllToAll converts head-sharding -> token-sharding: one collective for
    batch 0 (hidden under batch 1's attention), two half-collectives for
    batch 1 so only the last ~quarter is exposed
  - output projection with full Wo^T on the local token slices
proba is written to DRAM transposed and strip-blocked
([bh, strip, k, q_local]); the causal upper triangle is never written
(output buffers are pre-zeroed by the runtime); the host restores [q, k].
"""

import numpy as np

import concourse.bacc as bacc
import concourse.tile as tile
import concourse.mybir as mybir
import concourse.bass_utils as bass_utils
from concourse import masks

B, S, D, H = 2, 2048, 1024, 16
HD = D // H          # 64
N_CORES = 8
HPC = H // N_CORES   # heads per core = 2
CPC = HPC * HD       # channels per core = 128
T = B * S            # 4096 flattened tokens
SPC = S // N_CORES   # 256 tokens per core for the batch-0 a2a
HPT = S // 2 // N_CORES  # 128 tokens per core per batch-1 half a2a
STRIP = 512
N_STRIP = S // STRIP  # 4 strips per batch element
NJT = S // 128       # 16 k-tiles of 128

F32 = mybir.dt.float32
F32R = mybir.dt.float32r
EXP = mybir.ActivationFunctionType.Exp

_COMPILED = None


def _build():
    nc = bacc.Bacc(
        "TRN2",
        target_bir_lowering=False,
        debug=False,
        enable_asserts=True,
        num_devices=N_CORES,
    )
    # ---- I/O ----
    hst = nc.dram_tensor("hst", [D, T], F32, kind="ExternalInput").ap()       # hs^T
    wqt = nc.dram_tensor("wqt", [D, CPC], F32, kind="ExternalInput").ap()     # Wq_c^T
    wkt = nc.dram_tensor("wkt", [D, CPC], F32, kind="ExternalInput").ap()
    wvt = nc.dram_tensor("wvt", [D, CPC], F32, kind="ExternalInput").ap()
    wot = nc.dram_tensor("wot", [D, D], F32, kind="ExternalInput").ap()       # Wo^T
    bo = nc.dram_tensor("bo", [1, D], F32, kind="ExternalInput").ap()
    # out rows: [b0 tokens (256), b1-half0 (128), b1-half1 (128)]
    out = nc.dram_tensor("out", [B * SPC, D], F32, kind="ExternalOutput").ap()
    # proba[b*HPC + h][strip][k][q_local]  (transposed, strip-blocked)
    proba = nc.dram_tensor("proba", [B * HPC, N_STRIP, S, STRIP], F32,
                           kind="ExternalOutput").ap()

    hst_r = hst.bitcast(F32R)

    with tile.TileContext(nc) as tc:
        with tc.tile_pool(name="const", bufs=1) as constp, \
             tc.tile_pool(name="dram", bufs=1, space="DRAM") as dram:
            ident = constp.tile([128, 128], F32)
            masks.make_identity(nc, ident[:])
            ones_row = constp.tile([1, 128], F32R)
            nc.vector.memset(ones_row.bitcast(F32)[:], 1.0)

            wo_sb = constp.tile([128, 8 * D], F32R, name="wo_sb")
            bo_sb = constp.tile([1, D], F32)
            bo_b = constp.tile([128, D], F32)

            a2a0_in = dram.tile([N_CORES, CPC, SPC], F32, name="a2a0_in")
            a2a0_out = dram.tile([N_CORES, CPC, SPC], F32, name="a2a0_out")
            a2a1_in = [dram.tile([N_CORES, CPC, HPT], F32, name=f"a2a1i{u}")
                       for u in range(2)]
            a2a1_out = [dram.tile([N_CORES, CPC, HPT], F32, name=f"a2a1o{u}")
                        for u in range(2)]

            qt, kt, vaug = [], [], []
            with tc.tile_pool(name="qkvp", bufs=1) as qkv:
                for b in range(B):
                    qt.append(qkv.tile([128, S], F32R, name=f"qt{b}"))
                    kt.append(qkv.tile([128, S], F32R, name=f"kt{b}"))
                    vaug.append([qkv.tile([128, NJT * 65], F32R, name=f"vaug{b}{h}")
                                 for h in range(HPC)])
                # ---- projections for both batches ----
                with tc.tile_pool(name="hstp", bufs=8) as hsp, \
                     tc.tile_pool(name="prps", bufs=4, space="PSUM") as prps, \
                     tc.tile_pool(name="vtmp", bufs=2) as vtmp:
                    w_sb = vtmp.tile([128, 3 * 8 * CPC], F32R, name="w_sb", bufs=1)
                    for i, w in enumerate((wqt, wkt, wvt)):
                        nc.sync.dma_start(
                            w_sb[:, i * 8 * CPC:(i + 1) * 8 * CPC],
                            w.bitcast(F32R).rearrange("(c p) m -> p c m", p=128),
                        )
                    for b in range(B):
                        _proj_batch(nc, b, hst_r, w_sb, ident, hsp, prps, vtmp,
                                    qt[b], kt[b], vaug[b])
                    # Wo^T + bias loads: after proj, on the gpsimd queue
                    nc.gpsimd.dma_start(
                        wo_sb[:],
                        wot.bitcast(F32R).rearrange("(c p) m -> p c m", p=128),
                    )
                    nc.gpsimd.dma_start(bo_sb[:], bo[:])
                    nc.gpsimd.partition_broadcast(bo_b[:], bo_sb[:])

                # ---- attention + a2a + output projection ----
                with tc.tile_pool(name="ptp", bufs=10) as ptp, \
                     tc.tile_pool(name="smp", bufs=2) as smp, \
                     tc.tile_pool(name="mmps", bufs=1, space="PSUM") as mmps:
                    _attn(nc, 0, (0, 1, 2, 3), qt[0], kt[0], vaug[0], ones_row,
                          proba, ptp, smp, mmps,
                          lambda i4, h, ctx_sb: _ship_b0(nc, a2a0_in, i4, h, ctx_sb))
                    nc.gpsimd.collective_compute(
                        "AllToAll", mybir.AluOpType.bypass,
                        replica_groups=[list(range(N_CORES))],
                        ins=[a2a0_in.opt()], outs=[a2a0_out.opt()],
                    )
                    _attn(nc, 1, (0, 1), qt[1], kt[1], vaug[1], ones_row,
                          proba, ptp, smp, mmps,
                          lambda i4, h, ctx_sb: _ship_b1(nc, a2a1_in, i4, h, ctx_sb))
                    nc.gpsimd.collective_compute(
                        "AllToAll", mybir.AluOpType.bypass,
                        replica_groups=[list(range(N_CORES))],
                        ins=[a2a1_in[0].opt()], outs=[a2a1_out[0].opt()],
                    )
                    _attn(nc, 1, (2, 3), qt[1], kt[1], vaug[1], ones_row,
                          proba, ptp, smp, mmps,
                          lambda i4, h, ctx_sb: _ship_b1(nc, a2a1_in, i4, h, ctx_sb))
                    _outproj(nc, a2a0_out, SPC, 0, wo_sb, bo_b, out, smp, mmps)
                    nc.gpsimd.collective_compute(
                        "AllToAll", mybir.AluOpType.bypass,
                        replica_groups=[list(range(N_CORES))],
                        ins=[a2a1_in[1].opt()], outs=[a2a1_out[1].opt()],
                    )
                    _outproj(nc, a2a1_out[0], HPT, SPC, wo_sb, bo_b, out,
                             smp, mmps)
                    _outproj(nc, a2a1_out[1], HPT, SPC + HPT, wo_sb, bo_b, out,
                             smp, mmps)

    nc.compile()
    return nc


def _proj_batch(nc, b, hst_r, w_sb, ident, hsp, prps, vtmp, qt, kt, vaug):
    hs = []
    for kc in range(8):
        t_ = hsp.tile([128, S], F32R, tag="hst", name=f"hs{b}{kc}")
        nc.sync.dma_start(t_[:], hst_r[kc * 128:(kc + 1) * 128,
                                        b * S:(b + 1) * S])
        hs.append(t_)

    vt = vtmp.tile([128, S], F32, name=f"vt{b}", bufs=1)
    for i, dst in enumerate((qt, kt, vt)):
        for n in range(S // 512):
            ps = prps.tile([128, 512], F32, tag="prps", name=f"pp{b}")
            for kc in range(8):
                nc.tensor.matmul(
                    ps[:],
                    w_sb[:, (i * 8 + kc) * CPC:(i * 8 + kc + 1) * CPC],
                    hs[kc][:, n * 512:(n + 1) * 512],
                    start=(kc == 0),
                    stop=(kc == 7),
                )
            if i < 2:
                nc.scalar.copy(dst[:, n * 512:(n + 1) * 512], ps[:])
            else:
                nc.vector.tensor_copy(dst[:, n * 512:(n + 1) * 512], ps[:])

    # transpose v^T -> token-major v, pack into v_aug with ones col
    for h in range(HPC):
        nc.vector.memset(
            vaug[h].bitcast(F32).rearrange("p (j c) -> p j c", c=65)[:, :, 64],
            1.0,
        )
    for jt in range(NJT):
        pst = prps.tile([128, 128], F32, tag="pst", bufs=2, name=f"pst{b}")
        nc.tensor.transpose(pst[:], vt[:, jt * 128:(jt + 1) * 128], ident[:])
        for h in range(HPC):
            nc.vector.tensor_copy(
                vaug[h][:, jt * 65: jt * 65 + 64],
                pst[:, h * 64:(h + 1) * 64],
            )


def _ship_b0(nc, a2a_in, i4, h, ctx_sb):
    for half in range(2):  # strip covers two 256-token slices
        d = 2 * i4 + half
        nc.sync.dma_start(
            a2a_in[d, h * 64:(h + 1) * 64, :],
            ctx_sb[:, half * SPC:(half + 1) * SPC],
        )


def _ship_b1(nc, a2a_in, i4, h, ctx_sb):
    u = i4 // 2  # which half-collective
    for quarter in range(4):  # strip covers four 128-token slices
        d = 4 * (i4 % 2) + quarter
        nc.sync.dma_start(
            a2a_in[u][d, h * 64:(h + 1) * 64, :],
            ctx_sb[:, quarter * HPT:(quarter + 1) * HPT],
        )


def _attn(nc, b, strips, qt, kt, vaug, ones_row, proba, ptp, smp, mmps, ship):
    for i4 in strips:
        q0 = i4 * STRIP
        njs = (q0 + STRIP) // 128
        ngr = (njs + 3) // 4  # j-groups of 4 (each group = one pt tile)
        ctx_ps = [mmps.tile([65, STRIP], F32, tag=f"cps{h}", name=f"cp{b}{h}",
                            bufs=1) for h in range(HPC)]
        pts = [[None] * ngr for _ in range(HPC)]
        for j in range(njs):
            g, jj = j // 4, j % 4
            # causal trim: block j only has valid q >= 128j
            t0 = max(0, 128 * (j - 4 * i4))
            scs = []
            for h in range(HPC):  # adjacent emission -> PE row-group packing
                sc = mmps.tile([128, STRIP], F32, tag=f"sps{h}", name=f"sc{b}{h}",
                               bufs=3)
                nc.tensor.matmul(
                    sc[:, t0:],
                    kt[h * 64:(h + 1) * 64, j * 128:(j + 1) * 128],
                    qt[h * 64:(h + 1) * 64, q0 + t0:q0 + STRIP],
                    start=True, stop=True,
                    tile_position=(64 * h, 0),
                )
                scs.append(sc)
            for h in range(HPC):
                if jj == 0:
                    pts[h][g] = ptp.tile([128, 4 * STRIP], F32R, tag="pt",
                                         name=f"pt{b}{h}")
                pt = pts[h][g][:, jj * STRIP:(jj + 1) * STRIP]
                nc.scalar.activation(pt[:, t0:], scs[h][:, t0:], EXP)
                if j * 128 + 127 >= q0:  # diagonal 128-wide triangle
                    nc.gpsimd.affine_select(
                        out=pt[:, t0:t0 + 128], in_=pt[:, t0:t0 + 128],
                        compare_op=mybir.AluOpType.is_ge,
                        fill=0.0, base=0,
                        pattern=[[1, 128]],
                        channel_multiplier=-1,
                    )
            for h in range(HPC):
                nc.tensor.matmul(
                    ctx_ps[h][:, t0:],
                    vaug[h][:, j * 65:(j + 1) * 65],
                    pts[h][g][:, jj * STRIP + t0:(jj + 1) * STRIP],
                    start=(j == 0), stop=(j == njs - 1),
                )

        for h in range(HPC):
            # 1/denom on DVE (approx), then K=1 f32r PE broadcast
            den = smp.tile([1, STRIP], F32, tag="den", name=f"dn{b}{h}", bufs=2)
            nc.vector.tensor_copy(den[:], ctx_ps[h][64:65, :])
            rcp = smp.tile([1, STRIP], F32, tag="rcp", name=f"rf{b}{h}", bufs=2)
            nc.vector.reciprocal_approx_fast(out=rcp[:], in_=den[:])
            recip = smp.tile([1, STRIP], F32R, tag="recip", name=f"rc{b}{h}",
                             bufs=2)
            nc.vector.tensor_copy(recip[:], rcp[:])
            rb_ps = mmps.tile([128, STRIP], F32, tag=f"sps{h}", name=f"rp{b}{h}",
                              bufs=3)
            nc.tensor.matmul(rb_ps[:], ones_row[:], recip[:], start=True,
                             stop=True)
            rb_sb = smp.tile([128, STRIP], F32, tag="rb_sb", name=f"rs{b}{h}",
                             bufs=2)
            nc.vector.tensor_copy(rb_sb[:], rb_ps[:])
            bh = b * HPC + h
            # normalize + store P^T: full groups get one batched DMA,
            # the diagonal group keeps per-j trimmed writes
            for j in range(njs):
                g, jj = j // 4, j % 4
                t0 = max(0, 128 * (j - 4 * i4))
                pt = pts[h][g][:, jj * STRIP:(jj + 1) * STRIP]
                eng = nc.gpsimd if j % 3 == 2 else nc.vector
                eng.tensor_mul(pt[:, t0:], pt[:, t0:], rb_sb[:, t0:])
                if g == i4:  # diagonal group: trimmed per-j writes
                    nc.sync.dma_start(
                        proba[bh, i4, j * 128:(j + 1) * 128, t0:],
                        pt.bitcast(F32)[:, t0:],
                    )
                elif jj == 3:  # full group done: one batched write
                    nc.sync.dma_start(
                        proba[bh, i4].rearrange("(j p) q -> p j q", p=128)
                        [:, 4 * g:4 * g + 4, :],
                        pts[h][g].bitcast(F32)
                        .rearrange("p (j q) -> p j q", q=STRIP),
                    )
            # normalize ctx, ship into the a2a input slots
            ctx_sb = smp.tile([64, STRIP], F32, tag="ctx_sb", name=f"cs{b}{h}",
                              bufs=2)
            nc.vector.tensor_mul(ctx_sb[:], ctx_ps[h][0:64, :], rb_sb[0:64, :])
            ship(i4, h, ctx_sb)


def _outproj(nc, a2a_out, width, row0, wo_sb, bo_b, out, smp, mmps):
    ctxf = smp.tile([128, 8 * SPC], F32R, tag="ctxf", name=f"cf{row0}", bufs=1)
    for kc in range(N_CORES):
        nc.sync.dma_start(
            ctxf[:, kc * width:(kc + 1) * width],
            a2a_out[kc].bitcast(F32R),
        )
    for t in range(width // 128):
        for n in range(2):
            ps = mmps.tile([128, 512], F32, tag=f"sps{n}", name=f"op{row0}{n}",
                           bufs=3)
            for kc in range(8):
                nc.tensor.matmul(
                    ps[:],
                    ctxf[:, kc * width + t * 128: kc * width + (t + 1) * 128],
                    wo_sb[:, kc * D + n * 512: kc * D + (n + 1) * 512],
                    start=(kc == 0),
                    stop=(kc == 7),
                )
            ot = smp.tile([128, 512], F32, tag="osb_t", name=f"ot{row0}{n}",
                          bufs=2)
            nc.vector.tensor_add(ot[:], ps[:], bo_b[:, n * 512:(n + 1) * 512])
            nc.sync.dma_start(
                out[row0 + t * 128: row0 + (t + 1) * 128,
                    n * 512:(n + 1) * 512],
                ot[:],
            )


def _get_nc():
    global _COMPILED
    if _COMPILED is None:
        _COMPILED = _build()
    return _COMPILED


def _prep_in_maps(inputs):
    hidden_states = np.asarray(inputs["hidden_states"], dtype=np.float32)
    Wq = np.asarray(inputs["Wq"], dtype=np.float32)
    Wk = np.asarray(inputs["Wk"], dtype=np.float32)
    Wv = np.asarray(inputs["Wv"], dtype=np.float32)
    Wo = np.asarray(inputs["Wo"], dtype=np.float32)
    bo = np.asarray(inputs["bo"], dtype=np.float32)

    hst = np.ascontiguousarray(hidden_states.reshape(T, D).T)   # [D, T]
    wot = np.ascontiguousarray(Wo.T)                            # [D, D]
    bo2 = bo.reshape(1, D)
    in_maps = []
    for c in range(N_CORES):
        sl = slice(c * CPC, (c + 1) * CPC)
        in_maps.append({
            "hst": hst,
            "wqt": np.ascontiguousarray(Wq[sl, :].T),
            "wkt": np.ascontiguousarray(Wk[sl, :].T),
            "wvt": np.ascontiguousarray(Wv[sl, :].T),
            "wot": wot,
            "bo": bo2,
        })
    return in_maps


def kernel(hidden_states, attention_mask, Wq, Wk, Wv, Wo, bo):
    in_maps = _prep_in_maps({
        "hidden_states": hidden_states, "Wq": Wq, "Wk": Wk,
        "Wv": Wv, "Wo": Wo, "bo": bo,
    })
    nc = _get_nc()
    res = bass_utils.run_bass_kernel_spmd(
        nc, in_maps, core_ids=list(range(N_CORES))
    )

    # out rows per core: [b0 tokens 256c..] [b1 tokens 128c..] [b1 1024+128c..]
    output = np.empty((B, S, D), dtype=np.float32)
    for c in range(N_CORES):
        o = res.results[c]["out"]
        output[0, c * SPC:(c + 1) * SPC] = o[0:SPC]
        output[1, c * HPT:(c + 1) * HPT] = o[SPC:SPC + HPT]
        output[1, 1024 + c * HPT:1024 + (c + 1) * HPT] = o[SPC + HPT:]
    proba = np.empty((B, H, S, S), dtype=np.float32)
    for c in range(N_CORES):
        pr = res.results[c]["proba"]  # [B*HPC, N_STRIP, S(k), STRIP(q)]
        for b in range(B):
            for h in range(HPC):
                blk = pr[b * HPC + h]
                for i4 in range(N_STRIP):
                    proba[b, HPC * c + h, i4 * STRIP:(i4 + 1) * STRIP, :] = \
                        blk[i4].T
    return output, proba


# revision 19
# speedup vs baseline: 1.2251x; 1.0595x over previous
"""Distributed Trainium2 kernel for AutoRegressiveGlobalSelfAttention.

B=2, S=2048, D=1024, H=16 (head_dim 64), causal, no 1/sqrt(hd) scale.
Returns (output [B,S,D], proba [B,H,S,S]) like the reference nn.Module.

Sharding: tensor-parallel over heads. Core c owns heads {2c, 2c+1} for both
batch elements. Per core:
  - QKV projections for its 128 output channels (f32r matmuls on the PE)
  - attention in transposed layout S^T[k,q]: exp on ScalarE, causal diagonal
    via affine_select, softmax denominator via a ones-column appended to V
    (the context matmul computes row sums for free); blocks strictly above
    the diagonal are never computed (causal trim)
  - context comes out channel-major [ch, tok] == the AllToAll input layout
  - AllToAll converts head-sharding -> token-sharding: one collective for
    batch 0 (hidden under batch 1's attention), two half-collectives for
    batch 1 so only the last ~quarter is exposed
  - output projection with full Wo^T on the local token slices
proba is written to DRAM transposed and strip-blocked
([bh, strip, k, q_local]); the causal upper triangle is never written
(output buffers are pre-zeroed by the runtime); the host restores [q, k].
"""

import numpy as np

import concourse.bacc as bacc
import concourse.tile as tile
import concourse.mybir as mybir
import concourse.bass_utils as bass_utils
from concourse import masks

B, S, D, H = 2, 2048, 1024, 16
HD = D // H          # 64
N_CORES = 8
HPC = H // N_CORES   # heads per core = 2
CPC = HPC * HD       # channels per core = 128
T = B * S            # 4096 flattened tokens
SPC = S // N_CORES   # 256 tokens per core for the batch-0 a2a
HPT = S // 2 // N_CORES  # 128 tokens per core per batch-1 half a2a
STRIP = 512
N_STRIP = S // STRIP  # 4 strips per batch element
NJT = S // 128       # 16 k-tiles of 128

F32 = mybir.dt.float32
F32R = mybir.dt.float32r
EXP = mybir.ActivationFunctionType.Exp

_COMPILED = None


def _build():
    nc = bacc.Bacc(
        "TRN2",
        target_bir_lowering=False,
        debug=False,
        enable_asserts=True,
        num_devices=N_CORES,
    )
    # ---- I/O ----
    hst = nc.dram_tensor("hst", [D, T], F32, kind="ExternalInput").ap()       # hs^T
    wqt = nc.dram_tensor("wqt", [D, CPC], F32, kind="ExternalInput").ap()     # Wq_c^T
    wkt = nc.dram_tensor("wkt", [D, CPC], F32, kind="ExternalInput").ap()
    wvt = nc.dram_tensor("wvt", [D, CPC], F32, kind="ExternalInput").ap()
    wot = nc.dram_tensor("wot", [D, D], F32, kind="ExternalInput").ap()       # Wo^T
    bo = nc.dram_tensor("bo", [1, D], F32, kind="ExternalInput").ap()
    # out rows: [b0 tokens (256), b1-half0 (128), b1-half1 (128)]
    out = nc.dram_tensor("out", [B * SPC, D], F32, kind="ExternalOutput").ap()
    # proba[b*HPC + h][strip][k][q_local]  (transposed, strip-blocked)
    proba = nc.dram_tensor("proba", [B * HPC, N_STRIP, S, STRIP], F32,
                           kind="ExternalOutput").ap()

    hst_r = hst.bitcast(F32R)

    with tile.TileContext(nc) as tc:
        with tc.tile_pool(name="const", bufs=1) as constp, \
             tc.tile_pool(name="dram", bufs=1, space="DRAM") as dram:
            ident = constp.tile([128, 128], F32)
            masks.make_identity(nc, ident[:])
            ones_row = constp.tile([1, 128], F32R)
            nc.vector.memset(ones_row.bitcast(F32)[:], 1.0)

            wo_sb = constp.tile([128, 8 * D], F32R, name="wo_sb")
            bo_sb = constp.tile([1, D], F32)
            bo_b = constp.tile([128, D], F32)

            a2a0_in = dram.tile([N_CORES, CPC, SPC], F32, name="a2a0_in")
            a2a0_out = dram.tile([N_CORES, CPC, SPC], F32, name="a2a0_out")
            a2a1_in = [dram.tile([N_CORES, CPC, HPT], F32, name=f"a2a1i{u}")
                       for u in range(2)]
            a2a1_out = [dram.tile([N_CORES, CPC, HPT], F32, name=f"a2a1o{u}")
                        for u in range(2)]

            qt, kt, vaug = [], [], []
            with tc.tile_pool(name="qkvp", bufs=1) as qkv:
                for b in range(B):
                    qt.append(qkv.tile([128, S], F32R, name=f"qt{b}"))
                    kt.append(qkv.tile([128, S], F32R, name=f"kt{b}"))
                    vaug.append([qkv.tile([128, NJT * 65], F32R, name=f"vaug{b}{h}")
                                 for h in range(HPC)])
                # ---- projections for both batches ----
                with tc.tile_pool(name="hstp", bufs=8) as hsp, \
                     tc.tile_pool(name="prps", bufs=4, space="PSUM") as prps, \
                     tc.tile_pool(name="vtmp", bufs=2) as vtmp:
                    w_sb = vtmp.tile([128, 3 * 8 * CPC], F32R, name="w_sb", bufs=1)
                    for i, w in enumerate((wqt, wkt, wvt)):
                        nc.sync.dma_start(
                            w_sb[:, i * 8 * CPC:(i + 1) * 8 * CPC],
                            w.bitcast(F32R).rearrange("(c p) m -> p c m", p=128),
                        )
                    for b in range(B):
                        _proj_batch(nc, b, hst_r, w_sb, ident, hsp, prps, vtmp,
                                    qt[b], kt[b], vaug[b])
                    # Wo^T + bias loads: after proj, on the gpsimd queue
                    nc.gpsimd.dma_start(
                        wo_sb[:],
                        wot.bitcast(F32R).rearrange("(c p) m -> p c m", p=128),
                    )
                    nc.gpsimd.dma_start(bo_sb[:], bo[:])
                    nc.gpsimd.partition_broadcast(bo_b[:], bo_sb[:])

                # ---- attention + a2a + output projection ----
                with tc.tile_pool(name="ptp", bufs=10) as ptp, \
                     tc.tile_pool(name="smp", bufs=2) as smp, \
                     tc.tile_pool(name="mmps", bufs=1, space="PSUM") as mmps:
                    _attn(nc, 0, (0, 1, 2, 3), qt[0], kt[0], vaug[0], ones_row,
                          proba, ptp, smp, mmps,
                          lambda i4, h, ctx_sb: _ship_b0(nc, a2a0_in, i4, h, ctx_sb))
                    nc.gpsimd.collective_compute(
                        "AllToAll", mybir.AluOpType.bypass,
                        replica_groups=[list(range(N_CORES))],
                        ins=[a2a0_in.opt()], outs=[a2a0_out.opt()],
                    )
                    _attn(nc, 1, (0, 1), qt[1], kt[1], vaug[1], ones_row,
                          proba, ptp, smp, mmps,
                          lambda i4, h, ctx_sb: _ship_b1(nc, a2a1_in, i4, h, ctx_sb))
                    nc.gpsimd.collective_compute(
                        "AllToAll", mybir.AluOpType.bypass,
                        replica_groups=[list(range(N_CORES))],
                        ins=[a2a1_in[0].opt()], outs=[a2a1_out[0].opt()],
                    )
                    _attn(nc, 1, (2, 3), qt[1], kt[1], vaug[1], ones_row,
                          proba, ptp, smp, mmps,
                          lambda i4, h, ctx_sb: _ship_b1(nc, a2a1_in, i4, h, ctx_sb))
                    _outproj(nc, a2a0_out, SPC, 0, wo_sb, bo_b, out, smp, mmps)
                    nc.gpsimd.collective_compute(
                        "AllToAll", mybir.AluOpType.bypass,
                        replica_groups=[list(range(N_CORES))],
                        ins=[a2a1_in[1].opt()], outs=[a2a1_out[1].opt()],
                    )
                    _outproj(nc, a2a1_out[0], HPT, SPC, wo_sb, bo_b, out,
                             smp, mmps)
                    _outproj(nc, a2a1_out[1], HPT, SPC + HPT, wo_sb, bo_b, out,
                             smp, mmps)

    nc.compile()
    return nc


def _proj_batch(nc, b, hst_r, w_sb, ident, hsp, prps, vtmp, qt, kt, vaug):
    vt = vtmp.tile([128, S], F32, name=f"vt{b}", bufs=1)
    # n-granular streaming: each [128,512] hs tile is consumed by all three
    # projections right after it lands, so DMA and PE fully overlap
    for n in range(S // 512):
        hs = []
        for kc in range(8):
            t_ = hsp.tile([128, 512], F32R, tag="hst", name=f"hs{b}{kc}")
            nc.sync.dma_start(
                t_[:],
                hst_r[kc * 128:(kc + 1) * 128,
                      b * S + n * 512:b * S + (n + 1) * 512],
            )
            hs.append(t_)
        for i, dst in enumerate((qt, kt, vt)):
            ps = prps.tile([128, 512], F32, tag="prps", name=f"pp{b}")
            for kc in range(8):
                nc.tensor.matmul(
                    ps[:],
                    w_sb[:, (i * 8 + kc) * CPC:(i * 8 + kc + 1) * CPC],
                    hs[kc][:],
                    start=(kc == 0),
                    stop=(kc == 7),
                )
            if i < 2:
                nc.scalar.copy(dst[:, n * 512:(n + 1) * 512], ps[:])
            else:
                nc.vector.tensor_copy(dst[:, n * 512:(n + 1) * 512], ps[:])

    # transpose v^T -> token-major v, pack into v_aug with ones col
    for h in range(HPC):
        nc.vector.memset(
            vaug[h].bitcast(F32).rearrange("p (j c) -> p j c", c=65)[:, :, 64],
            1.0,
        )
    for jt in range(NJT):
        pst = prps.tile([128, 128], F32, tag="pst", bufs=2, name=f"pst{b}")
        nc.tensor.transpose(pst[:], vt[:, jt * 128:(jt + 1) * 128], ident[:])
        for h in range(HPC):
            nc.vector.tensor_copy(
                vaug[h][:, jt * 65: jt * 65 + 64],
                pst[:, h * 64:(h + 1) * 64],
            )


def _ship_b0(nc, a2a_in, i4, h, ctx_sb):
    for half in range(2):  # strip covers two 256-token slices
        d = 2 * i4 + half
        nc.sync.dma_start(
            a2a_in[d, h * 64:(h + 1) * 64, :],
            ctx_sb[:, half * SPC:(half + 1) * SPC],
        )


def _ship_b1(nc, a2a_in, i4, h, ctx_sb):
    u = i4 // 2  # which half-collective
    for quarter in range(4):  # strip covers four 128-token slices
        d = 4 * (i4 % 2) + quarter
        nc.sync.dma_start(
            a2a_in[u][d, h * 64:(h + 1) * 64, :],
            ctx_sb[:, quarter * HPT:(quarter + 1) * HPT],
        )


def _attn(nc, b, strips, qt, kt, vaug, ones_row, proba, ptp, smp, mmps, ship):
    for i4 in strips:
        q0 = i4 * STRIP
        njs = (q0 + STRIP) // 128
        ngr = (njs + 3) // 4  # j-groups of 4 (each group = one pt tile)
        ctx_ps = [mmps.tile([65, STRIP], F32, tag=f"cps{h}", name=f"cp{b}{h}",
                            bufs=1) for h in range(HPC)]
        pts = [[None] * ngr for _ in range(HPC)]
        for j in range(njs):
            g, jj = j // 4, j % 4
            # causal trim: block j only has valid q >= 128j
            t0 = max(0, 128 * (j - 4 * i4))
            scs = []
            for h in range(HPC):  # adjacent emission -> PE row-group packing
                sc = mmps.tile([128, STRIP], F32, tag=f"sps{h}", name=f"sc{b}{h}",
                               bufs=3)
                nc.tensor.matmul(
                    sc[:, t0:],
                    kt[h * 64:(h + 1) * 64, j * 128:(j + 1) * 128],
                    qt[h * 64:(h + 1) * 64, q0 + t0:q0 + STRIP],
                    start=True, stop=True,
                    tile_position=(64 * h, 0),
                )
                scs.append(sc)
            for h in range(HPC):
                if jj == 0:
                    pts[h][g] = ptp.tile([128, 4 * STRIP], F32R, tag="pt",
                                         name=f"pt{b}{h}")
                pt = pts[h][g][:, jj * STRIP:(jj + 1) * STRIP]
                nc.scalar.activation(pt[:, t0:], scs[h][:, t0:], EXP)
                if j * 128 + 127 >= q0:  # diagonal 128-wide triangle
                    nc.gpsimd.affine_select(
                        out=pt[:, t0:t0 + 128], in_=pt[:, t0:t0 + 128],
                        compare_op=mybir.AluOpType.is_ge,
                        fill=0.0, base=0,
                        pattern=[[1, 128]],
                        channel_multiplier=-1,
                    )
            for h in range(HPC):
                nc.tensor.matmul(
                    ctx_ps[h][:, t0:],
                    vaug[h][:, j * 65:(j + 1) * 65],
                    pts[h][g][:, jj * STRIP + t0:(jj + 1) * STRIP],
                    start=(j == 0), stop=(j == njs - 1),
                )

        for h in range(HPC):
            # 1/denom on DVE (approx), then K=1 f32r PE broadcast
            den = smp.tile([1, STRIP], F32, tag="den", name=f"dn{b}{h}", bufs=2)
            nc.vector.tensor_copy(den[:], ctx_ps[h][64:65, :])
            rcp = smp.tile([1, STRIP], F32, tag="rcp", name=f"rf{b}{h}", bufs=2)
            nc.vector.reciprocal_approx_fast(out=rcp[:], in_=den[:])
            recip = smp.tile([1, STRIP], F32R, tag="recip", name=f"rc{b}{h}",
                             bufs=2)
            nc.vector.tensor_copy(recip[:], rcp[:])
            rb_ps = mmps.tile([128, STRIP], F32, tag=f"sps{h}", name=f"rp{b}{h}",
                              bufs=3)
            nc.tensor.matmul(rb_ps[:], ones_row[:], recip[:], start=True,
                             stop=True)
            rb_sb = smp.tile([128, STRIP], F32, tag="rb_sb", name=f"rs{b}{h}",
                             bufs=2)
            nc.vector.tensor_copy(rb_sb[:], rb_ps[:])
            bh = b * HPC + h
            # normalize + store P^T: full groups get one batched mul + DMA,
            # the diagonal group keeps per-j trimmed ops
            rb4 = rb_sb.unsqueeze(1).broadcast_to([128, 4, STRIP])
            for g in range(ngr):
                if g < i4:  # full group
                    ptg = pts[h][g].rearrange("p (j q) -> p j q", q=STRIP)
                    eng = nc.vector if g % 2 == 0 else nc.gpsimd
                    eng.tensor_mul(ptg[:], ptg[:], rb4[:])
                    nc.sync.dma_start(
                        proba[bh, i4].rearrange("(j p) q -> p j q", p=128)
                        [:, 4 * g:4 * g + 4, :],
                        pts[h][g].bitcast(F32)
                        .rearrange("p (j q) -> p j q", q=STRIP),
                    )
                else:  # diagonal group: trimmed per-j
                    for jj in range(4):
                        j = 4 * g + jj
                        t0 = 128 * jj
                        pt = pts[h][g][:, jj * STRIP:(jj + 1) * STRIP]
                        eng = nc.vector if jj % 2 == 0 else nc.gpsimd
                        eng.tensor_mul(pt[:, t0:], pt[:, t0:], rb_sb[:, t0:])
                        nc.sync.dma_start(
                            proba[bh, i4, j * 128:(j + 1) * 128, t0:],
                            pt.bitcast(F32)[:, t0:],
                        )
            # normalize ctx, ship into the a2a input slots
            ctx_sb = smp.tile([64, STRIP], F32, tag="ctx_sb", name=f"cs{b}{h}",
                              bufs=2)
            nc.vector.tensor_mul(ctx_sb[:], ctx_ps[h][0:64, :], rb_sb[0:64, :])
            ship(i4, h, ctx_sb)


def _outproj(nc, a2a_out, width, row0, wo_sb, bo_b, out, smp, mmps):
    ctxf = smp.tile([128, 8 * SPC], F32R, tag="ctxf", name=f"cf{row0}", bufs=1)
    for kc in range(N_CORES):
        nc.sync.dma_start(
            ctxf[:, kc * width:(kc + 1) * width],
            a2a_out[kc].bitcast(F32R),
        )
    for t in range(width // 128):
        for n in range(2):
            ps = mmps.tile([128, 512], F32, tag=f"sps{n}", name=f"op{row0}{n}",
                           bufs=3)
            for kc in range(8):
                nc.tensor.matmul(
                    ps[:],
                    ctxf[:, kc * width + t * 128: kc * width + (t + 1) * 128],
                    wo_sb[:, kc * D + n * 512: kc * D + (n + 1) * 512],
                    start=(kc == 0),
                    stop=(kc == 7),
                )
            ot = smp.tile([128, 512], F32, tag="osb_t", name=f"ot{row0}{n}",
                          bufs=2)
            nc.vector.tensor_add(ot[:], ps[:], bo_b[:, n * 512:(n + 1) * 512])
            nc.sync.dma_start(
                out[row0 + t * 128: row0 + (t + 1) * 128,
                    n * 512:(n + 1) * 512],
                ot[:],
            )


def _get_nc():
    global _COMPILED
    if _COMPILED is None:
        _COMPILED = _build()
    return _COMPILED


def _prep_in_maps(inputs):
    hidden_states = np.asarray(inputs["hidden_states"], dtype=np.float32)
    Wq = np.asarray(inputs["Wq"], dtype=np.float32)
    Wk = np.asarray(inputs["Wk"], dtype=np.float32)
    Wv = np.asarray(inputs["Wv"], dtype=np.float32)
    Wo = np.asarray(inputs["Wo"], dtype=np.float32)
    bo = np.asarray(inputs["bo"], dtype=np.float32)

    hst = np.ascontiguousarray(hidden_states.reshape(T, D).T)   # [D, T]
    wot = np.ascontiguousarray(Wo.T)                            # [D, D]
    bo2 = bo.reshape(1, D)
    in_maps = []
    for c in range(N_CORES):
        sl = slice(c * CPC, (c + 1) * CPC)
        in_maps.append({
            "hst": hst,
            "wqt": np.ascontiguousarray(Wq[sl, :].T),
            "wkt": np.ascontiguousarray(Wk[sl, :].T),
            "wvt": np.ascontiguousarray(Wv[sl, :].T),
            "wot": wot,
            "bo": bo2,
        })
    return in_maps


def kernel(hidden_states, attention_mask, Wq, Wk, Wv, Wo, bo):
    in_maps = _prep_in_maps({
        "hidden_states": hidden_states, "Wq": Wq, "Wk": Wk,
        "Wv": Wv, "Wo": Wo, "bo": bo,
    })
    nc = _get_nc()
    res = bass_utils.run_bass_kernel_spmd(
        nc, in_maps, core_ids=list(range(N_CORES))
    )

    # out rows per core: [b0 tokens 256c..] [b1 tokens 128c..] [b1 1024+128c..]
    output = np.empty((B, S, D), dtype=np.float32)
    for c in range(N_CORES):
        o = res.results[c]["out"]
        output[0, c * SPC:(c + 1) * SPC] = o[0:SPC]
        output[1, c * HPT:(c + 1) * HPT] = o[SPC:SPC + HPT]
        output[1, 1024 + c * HPT:1024 + (c + 1) * HPT] = o[SPC + HPT:]
    proba = np.empty((B, H, S, S), dtype=np.float32)
    for c in range(N_CORES):
        pr = res.results[c]["proba"]  # [B*HPC, N_STRIP, S(k), STRIP(q)]
        for b in range(B):
            for h in range(HPC):
                blk = pr[b * HPC + h]
                for i4 in range(N_STRIP):
                    proba[b, HPC * c + h, i4 * STRIP:(i4 + 1) * STRIP, :] = \
                        blk[i4].T
    return output, proba
